# revision 1
# baseline (speedup 1.0000x reference)
"""MoE routing kernel for Trainium2 (8 NeuronCores, Bass/Tile).

Problem: B=4, S=2048, D=1024, E=8, top_k=2.
  logits = x @ gate_w + gate_b          [B,S,E]
  mask   = one_hot(top2(logits)).sum    [B,S,E]   (binary - probs never used)
  y      = sum_e mask_e * (x @ W_e + b_e)

Strategy: token-data-parallel over 8 cores (1024 tokens/core), dense
per-expert matmuls with output masking.
  - host: shard tokens, pre-transpose each x-shard to xT [D, T] (layout prep
    only - the PE contracts over the partition dim, so both matmul operands
    need d on partitions)
  - device per core: fp32 gating matmul (exact top-2 vs the fp32 reference),
    top-2 mask via MAX8, then for each expert a full [T,D]x[D,D] matmul in
    float32r (full PE rate at N=512) masked-accumulated into y.
"""

import os
import sys

import numpy as np

sys.path.insert(0, "/opt/trn_rl_repo")

import concourse.bass as bass
import concourse.mybir as mybir
import concourse.tile as tile
from concourse import bacc
from concourse.bass_utils import run_bass_kernel_spmd
from concourse.masks import make_identity

P = 128
D = 1024
E = 8
N_CORES = 8
TOK_TOTAL = 8192
T_SHARD = TOK_TOTAL // N_CORES  # 1024 tokens per core
NT = T_SHARD // P  # 8 token tiles
ND = D // P  # 8 contraction tiles
FC = 2  # two 512-wide f chunks (one PSUM bank each)
FW = 512

F32 = mybir.dt.float32
F32R = mybir.dt.float32r

LAST_EXEC_TIME_NS = None


def _build_nc(repeat=1):
    nc = bacc.Bacc(None, target_bir_lowering=False)

    xt_r = nc.dram_tensor("xt_r", [D, T_SHARD], F32R, kind="ExternalInput")
    xt_g = nc.dram_tensor("xt_g", [D, T_SHARD], F32, kind="ExternalInput")
    gw = nc.dram_tensor("gw", [D, E], F32, kind="ExternalInput")
    gb = nc.dram_tensor("gb", [1, E], F32, kind="ExternalInput")
    ew = nc.dram_tensor("ew", [E, D, D], F32R, kind="ExternalInput")
    eb = nc.dram_tensor("eb", [E, D], F32, kind="ExternalInput")
    y = nc.dram_tensor("y", [T_SHARD, D], F32, kind="ExternalOutput")

    with tile.TileContext(nc) as tc:
        with (
            tc.tile_pool(name="const", bufs=1) as const_pool,
            tc.tile_pool(name="xpool", bufs=1) as xpool,
            tc.tile_pool(name="wpool", bufs=2) as wpool,
            tc.tile_pool(name="accpool", bufs=1) as accpool,
            tc.tile_pool(name="small", bufs=2) as small,
            tc.tile_pool(name="tmppool", bufs=4) as tmppool,
            tc.tile_pool(name="pg", bufs=1, space="PSUM") as pg_pool,
            tc.tile_pool(name="pe", bufs=6, space="PSUM") as pe_pool,
            tc.tile_pool(name="pt", bufs=1, space="PSUM") as pt_pool,
        ):
            identity = const_pool.tile([P, P], F32)
            make_identity(nc, identity[:])
            ones_row = const_pool.tile([1, P], F32)
            nc.vector.memset(ones_row[:], 1.0)

            gb_sb = const_pool.tile([1, E], F32)
            nc.sync.dma_start(out=gb_sb[:], in_=gb[:])
            eb_sb = const_pool.tile([E, D], F32)
            nc.sync.dma_start(out=eb_sb[:], in_=eb[:])

            gate_sb = []
            for dt in range(ND):
                g_t = const_pool.tile([P, E], F32, name=f"g_{dt}")
                nc.sync.dma_start(out=g_t[:], in_=gw[dt * P : (dt + 1) * P, :])
                gate_sb.append(g_t)

            # x^T, resident for the whole kernel: f32r copy for expert matmuls,
            # f32 copy for exact gating.
            xtr_sb = []
            xtg_sb = []
            for dt in range(ND):
                xr_t = xpool.tile([P, T_SHARD], F32R, name=f"xr_{dt}")
                nc.sync.dma_start(out=xr_t[:], in_=xt_r[dt * P : (dt + 1) * P, :])
                xtr_sb.append(xr_t)
                xg_t = xpool.tile([P, T_SHARD], F32, name=f"xg_{dt}")
                nc.sync.dma_start(out=xg_t[:], in_=xt_g[dt * P : (dt + 1) * P, :])
                xtg_sb.append(xg_t)

            def body():
                # ---- gating: logits -> top-2 mask, all in exact fp32 ----
                mask_sb = []
                maskT_sb = []
                for tt in range(NT):
                    ts = slice(tt * P, (tt + 1) * P)
                    psum_g = pg_pool.tile([P, E], F32, name="psum_g")
                    for dt in range(ND):
                        nc.tensor.matmul(
                            out=psum_g[:],
                            lhsT=xtg_sb[dt][:, ts],
                            rhs=gate_sb[dt][:],
                            start=(dt == 0),
                            stop=False,
                        )
                    # + gate_b broadcast along tokens (K=1 matmul)
                    nc.tensor.matmul(
                        out=psum_g[:],
                        lhsT=ones_row[:],
                        rhs=gb_sb[:],
                        start=False,
                        stop=True,
                    )
                    logits = small.tile([P, E], F32, name="logits")
                    nc.vector.tensor_copy(out=logits[:], in_=psum_g[:])
                    max8 = small.tile([P, E], F32, name="max8")
                    nc.vector.max(out=max8[:], in_=logits[:])
                    m_t = small.tile([P, E], F32, name=f"mask_{tt}", bufs=1)
                    nc.vector.tensor_tensor(
                        out=m_t[:],
                        in0=logits[:],
                        in1=max8[:, 1:2].to_broadcast([P, E]),
                        op=mybir.AluOpType.is_ge,
                    )
                    mask_sb.append(m_t)
                    # mask^T (for the expert-bias term)
                    pt = pt_pool.tile([E, P], F32, name="pt")
                    nc.tensor.transpose(
                        out=pt[:], in_=m_t[:], identity=identity[:]
                    )
                    mT = small.tile([E, P], F32, name=f"maskT_{tt}", bufs=1)
                    nc.vector.tensor_copy(out=mT[:], in_=pt[:])
                    maskT_sb.append(mT)

                # ---- experts: dense matmul + masked accumulate ----
                acc_sb = [
                    accpool.tile([P, D], F32, name=f"acc_{tt}")
                    for tt in range(NT)
                ]

                for e in range(E):
                    w_sb = []
                    for dt in range(ND):
                        w_t = wpool.tile([P, D], F32R, name=f"w_{dt}")
                        nc.sync.dma_start(
                            out=w_t[:], in_=ew[e, dt * P : (dt + 1) * P, :]
                        )
                        w_sb.append(w_t)
                    for tt in range(NT):
                        ts = slice(tt * P, (tt + 1) * P)
                        for fc in range(FC):
                            fs = slice(fc * FW, (fc + 1) * FW)
                            ps = pe_pool.tile([P, FW], F32, name="ps")
                            for dt in range(ND):
                                nc.tensor.matmul(
                                    out=ps[:],
                                    lhsT=xtr_sb[dt][:, ts],
                                    rhs=w_sb[dt][:, fs],
                                    start=(dt == 0),
                                    stop=(dt == ND - 1),
                                )
                            m_bc = mask_sb[tt][:, e : e + 1].to_broadcast([P, FW])
                            if e == 0:
                                nc.vector.tensor_tensor(
                                    out=acc_sb[tt][:, fs],
                                    in0=ps[:],
                                    in1=m_bc,
                                    op=mybir.AluOpType.mult,
                                )
                            else:
                                tmp = tmppool.tile([P, FW], F32, name="tmp")
                                nc.vector.tensor_tensor(
                                    out=tmp[:],
                                    in0=ps[:],
                                    in1=m_bc,
                                    op=mybir.AluOpType.mult,
                                )
                                nc.gpsimd.tensor_tensor(
                                    out=acc_sb[tt][:, fs],
                                    in0=acc_sb[tt][:, fs],
                                    in1=tmp[:],
                                    op=mybir.AluOpType.add,
                                )

                # ---- + mask @ expert_b, then store ----
                for tt in range(NT):
                    for fc in range(FC):
                        fs = slice(fc * FW, (fc + 1) * FW)
                        pb = pe_pool.tile([P, FW], F32, name="ps")
                        nc.tensor.matmul(
                            out=pb[:],
                            lhsT=maskT_sb[tt][:],
                            rhs=eb_sb[:, fs],
                            start=True,
                            stop=True,
                        )
                        nc.vector.tensor_add(
                            out=acc_sb[tt][:, fs],
                            in0=acc_sb[tt][:, fs],
                            in1=pb[:],
                        )
                    nc.sync.dma_start(
                        out=y[tt * P : (tt + 1) * P, :], in_=acc_sb[tt][:]
                    )

            if repeat == 1:
                body()
            else:
                with tc.For_i(0, repeat, 1) as _i:
                    body()

    nc.compile()
    return nc


_NC_CACHE = {}


def _get_nc(repeat=1):
    if repeat not in _NC_CACHE:
        _NC_CACHE[repeat] = _build_nc(repeat)
    return _NC_CACHE[repeat]


def _make_in_maps(x, gate_w, gate_b, expert_w, expert_b):
    xf = x.reshape(TOK_TOTAL, D)
    in_maps = []
    for c in range(N_CORES):
        shard = xf[c * T_SHARD : (c + 1) * T_SHARD, :]
        xt = np.ascontiguousarray(shard.T)  # [D, T]
        in_maps.append(
            {
                "xt_r": xt,
                "xt_g": xt,
                "gw": gate_w,
                "gb": gate_b,
                "ew": expert_w,
                "eb": expert_b,
            }
        )
    return in_maps


def kernel(x, gate_w, gate_b, expert_w, expert_b, top_k):
    global LAST_EXEC_TIME_NS
    x = np.ascontiguousarray(np.asarray(x, dtype=np.float32))
    gate_w = np.ascontiguousarray(np.asarray(gate_w, dtype=np.float32))
    gate_b = np.asarray(gate_b, dtype=np.float32).reshape(1, E)
    expert_w = np.ascontiguousarray(np.asarray(expert_w, dtype=np.float32))
    expert_b = np.ascontiguousarray(np.asarray(expert_b, dtype=np.float32))
    assert int(top_k) == 2, "kernel is specialized for top_k=2"

    B, S, D_ = x.shape
    assert (B * S, D_) == (TOK_TOTAL, D)

    nc = _get_nc(1)
    in_maps = _make_in_maps(x, gate_w, gate_b, expert_w, expert_b)
    res = run_bass_kernel_spmd(nc, in_maps, core_ids=list(range(N_CORES)))
    LAST_EXEC_TIME_NS = res.exec_time_ns

    out = np.empty((TOK_TOTAL, D), dtype=np.float32)
    for c in range(N_CORES):
        out[c * T_SHARD : (c + 1) * T_SHARD, :] = res.results[c]["y"]
    return out.reshape(B, S, D)


# ---------------------------------------------------------------------------
# Timing support (test.py only). NTFF profiling is unavailable under this
# axon setup, so device time is measured by wall-clocking NEFFs that run the
# kernel body `repeat` times in an on-device For_i loop, with all operands
# device-resident, and differencing two repeat counts.
# ---------------------------------------------------------------------------


def _run_timed(nc, in_maps, n_timed=3):
    import time

    import jax
    from jax.experimental.shard_map import shard_map
    from jax.sharding import Mesh, NamedSharding, PartitionSpec

    import concourse.mybir as mybir_
    from concourse.bass2jax import (
        _bass_exec_p,
        install_neuronx_cc_hook,
        partition_id_tensor,
    )

    install_neuronx_cc_hook()
    partition_name = nc.partition_id_tensor.name if nc.partition_id_tensor else None
    in_names, out_names, out_avals, zero_outs = [], [], [], []
    for alloc in nc.m.functions[0].allocations:
        if not isinstance(alloc, mybir_.MemoryLocationSet):
            continue
        name = alloc.memorylocations[0].name
        if alloc.kind == "ExternalInput":
            if name != partition_name:
                in_names.append(name)
        elif alloc.kind == "ExternalOutput":
            shape = tuple(alloc.tensor_shape)
            dtype = mybir_.dt.np(alloc.dtype)
            out_avals.append(jax.core.ShapedArray(shape, dtype))
            out_names.append(name)
            zero_outs.append(np.zeros(shape, dtype))
    n_params = len(in_names)
    n_outs = len(out_avals)
    in_names = in_names + out_names
    if partition_name is not None:
        in_names.append(partition_name)

    def _body(*args):
        ops = list(args)
        if partition_name is not None:
            ops.append(partition_id_tensor())
        outs = _bass_exec_p.bind(
            *ops,
            out_avals=tuple(out_avals),
            in_names=tuple(in_names),
            out_names=tuple(out_names),
            lowering_input_output_aliases=(),
            sim_require_finite=True,
            sim_require_nnan=True,
            nc=nc,
        )
        return tuple(outs)

    devices = jax.devices()[:N_CORES]
    mesh = Mesh(np.asarray(devices), ("core",))
    in_specs = (PartitionSpec("core"),) * (n_params + n_outs)
    out_specs = (PartitionSpec("core"),) * n_outs
    fn = jax.jit(
        shard_map(
            _body, mesh=mesh, in_specs=in_specs, out_specs=out_specs, check_rep=False
        ),
        donate_argnums=tuple(range(n_params, n_params + n_outs)),
        keep_unused=True,
    )
    sharding = NamedSharding(mesh, PartitionSpec("core"))
    dev_in = [
        jax.device_put(
            np.concatenate(
                [np.asarray(in_maps[c][nm]) for c in range(N_CORES)], axis=0
            ),
            sharding,
        )
        for nm in in_names[:n_params]
    ]
    jax.block_until_ready(dev_in)

    def fresh_zeros():
        zs = [
            jax.device_put(
                np.zeros((N_CORES * z.shape[0], *z.shape[1:]), z.dtype), sharding
            )
            for z in zero_outs
        ]
        jax.block_until_ready(zs)
        return zs

    # warmup (compile + first exec)
    out = fn(*dev_in, *fresh_zeros())
    jax.block_until_ready(out)
    times = []
    for _ in range(n_timed):
        zs = fresh_zeros()
        t0 = time.perf_counter()
        out = fn(*dev_in, *zs)
        jax.block_until_ready(out)
        times.append(time.perf_counter() - t0)
    return times


def measure_exec_time_ns(inputs, rep_hi=65):
    x = np.ascontiguousarray(np.asarray(inputs["x"], dtype=np.float32))
    gate_w = np.ascontiguousarray(np.asarray(inputs["gate_w"], dtype=np.float32))
    gate_b = np.asarray(inputs["gate_b"], dtype=np.float32).reshape(1, E)
    expert_w = np.ascontiguousarray(np.asarray(inputs["expert_w"], dtype=np.float32))
    expert_b = np.ascontiguousarray(np.asarray(inputs["expert_b"], dtype=np.float32))
    in_maps = _make_in_maps(x, gate_w, gate_b, expert_w, expert_b)
    t_lo = _run_timed(_get_nc(1), in_maps, n_timed=5)
    t_hi = _run_timed(_get_nc(rep_hi), in_maps, n_timed=5)
    med = lambda ts: sorted(ts)[len(ts) // 2]
    per_iter_s = (med(t_hi) - med(t_lo)) / (rep_hi - 1)
    return per_iter_s * 1e9, t_lo, t_hi

